# revision 34
# baseline (speedup 1.0000x reference)
import sys

for p in ("/opt/trn_rl_repo",):
    if p not in sys.path:
        sys.path.insert(0, p)

import numpy as np

# Persistent XLA compilation cache: run_bass_kernel_spmd re-jits a fresh
# closure on every call, so without this every call (including warm ones)
# pays a full XLA+neuronx recompile (~115 ms). With the cache the repeat
# compile is a disk hit.
try:
    import jax

    jax.config.update("jax_compilation_cache_dir", "/tmp/jax_pcache")
    jax.config.update("jax_persistent_cache_min_compile_time_secs", 0.0)
    jax.config.update("jax_persistent_cache_min_entry_size_bytes", -1)
except Exception:
    pass

# Problem constants (hardcoded per contract)
B, F, E, U, H = 4096, 39, 64, 256, 8
DH = U // H               # 32 head dim
NCORES = 8
BC = B // NCORES          # 512 samples per core
T = BC * F                # 19968 tokens per core
SPB = 8                   # samples per block
NBLK = BC // SPB          # 64 blocks
TB = SPB * F              # 312 tokens per block
NPAIR = SPB // 2          # 4 sample-pairs per block
KA = E + 1                # contraction dim with bias row
VA = DH + 1               # v cols per head (+1 denominator ones-column)
UV = H * VA               # 264
GPB = TB // 8             # value-groups per block (39): 8 int6 values -> 6 bytes
GB = 6                    # packed bytes per group
PB = GPB * GB             # packed bytes per partition per block (234)
QMAX = 31                 # int6 symmetric range
CLIP_Q = 0.999            # clip quantile for the emb quantizer (~3.3 sigma)
# unpack plane table: value j sits at bit offset 6j of its 48-bit group ->
# (byte a, shift o); planes with o <= 2 fit in one byte
_PLANES = [(0, 0), (0, 6), (1, 4), (2, 2), (3, 0), (3, 6), (4, 4), (5, 2)]
# single-input byte layout: packed embeddings first, then aux fp32 [128,6],
# Wp fp16 [F,UV], W2 int8 [KA, 2U+UV]. Offsets derive from the emb length
# (fp32/fp16 sections stay 4/2-aligned: emb and aux sizes are multiples of 4)
AUX_BYTES = 128 * 6 * 4
WP_BYTES = F * UV * 2
W2_BYTES = KA * (2 * U + UV)


def _blob_offsets(n_blocks):
    emb_bytes = E * n_blocks * PB
    aux_off = emb_bytes
    wp_off = aux_off + AUX_BYTES
    w2_off = wp_off + WP_BYTES
    return emb_bytes, aux_off, wp_off, w2_off, w2_off + W2_BYTES
SCALE = 1.0 / float(np.sqrt(np.float32(DH)))
# Weights travel as int8 and are dequantized on device by a runtime scale
# (aux col 0/1). KQK/KV are static power-of-two boosts that keep the
# dequantized fp16 weights out of the subnormal range; KQK cancels in the
# exp scale, KV cancels via the ones_val column (aux cols 2/3).
KQK = 256.0
KV = 256.0
EXP_SCALE = SCALE / (KQK * KQK)

_CACHE = {}


def _build_program(n_blocks=NBLK, max_unroll=1):
    import concourse.bacc as bacc
    import concourse.bass as bass
    import concourse.mybir as mybir
    from concourse.tile import TileContext

    fp32 = mybir.dt.float32
    fp16 = mybir.dt.float16
    i8 = mybir.dt.int8
    Relu = mybir.ActivationFunctionType.Relu
    Exp = mybir.ActivationFunctionType.Exp
    Mult = mybir.AluOpType.mult
    Add = mybir.AluOpType.add
    Max = mybir.AluOpType.max
    Sub = mybir.AluOpType.subtract
    Shl = mybir.AluOpType.logical_shift_left
    Shr = mybir.AluOpType.logical_shift_right
    And = mybir.AluOpType.bitwise_and
    Or = mybir.AluOpType.bitwise_or
    Xor = mybir.AluOpType.bitwise_xor
    AxX = mybir.AxisListType.X

    nc = bacc.Bacc(None, target_bir_lowering=False)
    # ONE input per core: int6-bitstream embeddings (8 tokens -> 6 bytes,
    # unpacked on DVE per block) followed by the byte-packed weights
    # [aux fp32 [128,6] | Wp fp16 [F,UV] | W2 i8 [KA, 2U+UV]]
    # (W2 = Wqk|Wva column-concatenated)
    emb_bytes, aux_off, wp_off, w2_off, nb_all = _blob_offsets(n_blocks)
    wb = nc.dram_tensor("IN", (1, nb_all), i8, kind="ExternalInput")
    emb = wb[0, 0:emb_bytes].rearrange("(p c) -> p c", c=n_blocks * PB)
    out = nc.dram_tensor("out", (2, n_blocks * NPAIR), fp32, kind="ExternalOutput")

    with TileContext(nc) as tc:
        with (
            tc.tile_pool(name="const", bufs=1) as cp,
            tc.tile_pool(name="xin", bufs=3) as xp,
            tc.tile_pool(name="qk", bufs=2) as qkpool,
            tc.tile_pool(name="attn", bufs=2) as ap,
            tc.tile_pool(name="qkps", bufs=1, space="PSUM") as qkps,
            tc.tile_pool(name="vps", bufs=1, space="PSUM") as vpsp,
            tc.tile_pool(name="sps", bufs=1, space="PSUM") as spsp,
            tc.tile_pool(name="ops", bufs=1, space="PSUM") as opsp,
        ):
            # --- constants / weights (persistent) ---
            # aux fp32 [128, 6]: col0 = qk dequant scale, col1 = v dequant
            # scale, col2/3 = ones_val columns for the final per-block
            # reduction matmul (restores the KV/denominator scaling), col4 =
            # output bias bp (rows 0-1), col5 = Wp dequant scale.
            aux_sb = cp.tile([128, 6], fp32)
            nc.sync.dma_start(
                out=aux_sb[:],
                in_=wb[0, aux_off:aux_off + AUX_BYTES]
                .bitcast(fp32)
                .rearrange("(p c) -> p c", c=6),
            )
            w2_i8 = cp.tile([KA, 2 * U + UV], i8)
            nc.sync.dma_start(
                out=w2_i8[:],
                in_=wb[0, w2_off:w2_off + W2_BYTES]
                .rearrange("(p c) -> p c", c=2 * U + UV),
            )
            wqk_sb = cp.tile([KA, 2 * U], fp16)
            nc.scalar.mul(wqk_sb[:], w2_i8[:, 0:2 * U], aux_sb[0:KA, 0:1])
            wv_sb = cp.tile([KA, UV], fp16)
            nc.scalar.mul(wv_sb[:], w2_i8[:, 2 * U:2 * U + UV], aux_sb[0:KA, 1:2])
            wp_sb = cp.tile([128, H, VA], fp16)
            for base in (0, 64):
                nc.sync.dma_start(
                    out=wp_sb[base:base + F],
                    in_=wb[0, wp_off:wp_off + WP_BYTES]
                    .bitcast(fp16)
                    .rearrange("(p h v) -> p h v", p=F, v=VA),
                )
            logblk = cp.tile([128, NPAIR], fp32)
            nc.gpsimd.memset(logblk[:], 0.0)
            logits_sb = cp.tile([2, n_blocks * NPAIR], fp32)

            def block_body(i):
                # i: block index (python int or loop register)
                xi = xp.tile([E, PB], i8, tag="xi")
                nc.sync.dma_start(out=xi[:], in_=emb[:, bass.ds(i * PB, PB)])
                x = xp.tile([KA, TB], fp16, tag="x")
                # int6 bitstream -> fp16: value j of each 8-token group lives
                # at bits [6j, 6j+6) of its 6-byte group. NOTE Shr
                # sign-extends on int8 lanes, so every shifted term is
                # masked in the same instruction. Sign-extend 6-bit two's
                # complement via (v ^ 0x20) - 32.
                bplane = xi[:].rearrange("p (g k) -> p k g", k=GB)
                xplane = x[0:E, :].rearrange("p (g k) -> p k g", k=8)
                t2 = xp.tile([E, GPB], i8, tag="u2")
                t3 = xp.tile([E, GPB], i8, tag="u3")
                t4 = xp.tile([E, GPB], i8, tag="u4")
                t5 = xp.tile([E, GPB], i8, tag="u5")
                for j, (a, o) in enumerate(_PLANES):
                    if o == 0:
                        nc.vector.tensor_scalar(
                            t4[:], bplane[:, a, :], 0x3F, 0x20, And, Xor
                        )
                    elif o <= 2:
                        nc.vector.tensor_scalar(
                            t5[:], bplane[:, a, :], o, 0x3F, Shr, And
                        )
                        nc.vector.tensor_scalar(
                            t4[:], t5[:], 0x20, None, Xor
                        )
                    else:
                        nc.vector.tensor_scalar(
                            t2[:], bplane[:, a + 1, :], 8 - o, None, Shl
                        )
                        nc.vector.tensor_scalar(
                            t3[:], bplane[:, a, :], o, (1 << (8 - o)) - 1, Shr, And
                        )
                        nc.vector.tensor_tensor(
                            out=t5[:], in0=t3[:], in1=t2[:], op=Or
                        )
                        nc.vector.tensor_scalar(
                            t4[:], t5[:], 0x3F, 0x20, And, Xor
                        )
                    nc.vector.tensor_scalar(
                        xplane[:, j, :], t4[:], 32, None, Sub
                    )
                nc.gpsimd.memset(x[E:KA, :], 1.0)

                # q/k projections: psum [128, TB] per 128-wide u-slice
                qk_sb = []
                for m in range(4):
                    ps = qkps.tile([128, TB], fp32, tag="qkps")
                    nc.tensor.matmul(
                        ps[:], wqk_sb[:, m * 128:(m + 1) * 128], x[:],
                        start=True, stop=True,
                    )
                    sb = qkpool.tile([128, TB], fp16, tag=f"qk{m}")
                    nc.scalar.activation(sb[:], ps[:], Relu)
                    qk_sb.append(sb)
                q_lo, q_hi, k_lo, k_hi = qk_sb

                for p_ in range(NPAIR):
                    pair = ((2 * p_, 0), (2 * p_ + 1, 64))
                    # v projection (token-major, +ones col per head), one psum
                    # tile per sample; K=65 -> all v MMs share row groups 0-2
                    # (serialized on PE), outputs at partition base 0.
                    vt_s = []
                    for si, (a, base) in enumerate(pair):
                        vps = vpsp.tile([F, H, VA], fp32, tag=f"vps{si}")
                        nc.tensor.matmul(
                            vps[:, :, :].rearrange("p h v -> p (h v)"),
                            x[:, a * F:(a + 1) * F], wv_sb[:],
                            start=True, stop=True,
                        )
                        vt = ap.tile([F, H, VA], fp16, tag=f"vt{si}")
                        nc.scalar.activation(vt[:], vps[:], Relu)
                        vt_s.append(vt)

                    # scores^T = k q^T: one psum bank per PE row group
                    # (heads h and h+4 share a row group -> serialized, safe)
                    sgrp = []
                    for rg in range(4):
                        sg = spsp.tile([F, 2, 2, F], fp32, tag=f"sg{rg}")
                        sgrp.append(sg)
                    for si, (a, base) in enumerate(pair):
                        for h in range(H):
                            kt, qt = (k_lo, q_lo) if h < 4 else (k_hi, q_hi)
                            rg, hh = h % 4, h // 4
                            hb = rg * DH
                            nc.tensor.matmul(
                                sgrp[rg][:, hh, si, :],
                                kt[hb:hb + DH, a * F:(a + 1) * F],
                                qt[hb:hb + DH, a * F:(a + 1) * F],
                                start=True, stop=True,
                                tile_position=(hb, 0),
                            )
                    exg = []
                    for rg in range(4):
                        ex = ap.tile([F, 2, 2, F], fp16, tag=f"ex{rg}")
                        nc.scalar.activation(ex[:], sgrp[rg][:], Exp, scale=EXP_SCALE)
                        exg.append(ex)

                    # out_unnorm = exp^T @ v_aug (last col = denominator).
                    # K=39 -> all AV MMs share row groups 0-1 (serialized), so
                    # packing samples at partition bases 0/64 of one bank is safe.
                    ops_ = opsp.tile([128, H, VA], fp32, tag="opsum")
                    for si, (a, base) in enumerate(pair):
                        for h in range(H):
                            rg, hh = h % 4, h // 4
                            nc.tensor.matmul(
                                ops_[base:base + F, h, :],
                                exg[rg][:, hh, si, :],
                                vt_s[si][:, h, :],
                                start=True, stop=True,
                            )
                    rc = ap.tile([128, H], fp32, tag="rc")
                    t = ap.tile([128, H, DH], fp32, tag="t")
                    part = ap.tile([128, H], fp32, tag="part")
                    t2 = ap.tile([128, H], fp32, tag="t2")
                    for base in (0, 64):
                        nc.vector.reciprocal(
                            rc[base:base + F], ops_[base:base + F, :, DH]
                        )
                        # t = relu(out_unnorm) * Wp   (relu+mul fused)
                        nc.vector.scalar_tensor_tensor(
                            out=t[base:base + F],
                            in0=ops_[base:base + F, :, 0:DH],
                            scalar=0.0,
                            in1=wp_sb[base:base + F, :, 0:DH],
                            op0=Max,
                            op1=Mult,
                        )
                        # partial[f, h] = sum_d t
                        nc.vector.tensor_reduce(
                            out=part[base:base + F],
                            in_=t[base:base + F],
                            axis=AxX,
                            op=Add,
                        )
                        # logit partials per f-row: sum_h partial * (1/denom)
                        nc.vector.tensor_mul(
                            t2[base:base + F], part[base:base + F], rc[base:base + F]
                        )
                        nc.vector.tensor_reduce(
                            out=logblk[base:base + F, p_:p_ + 1],
                            in_=t2[base:base + F],
                            axis=AxX,
                            op=Add,
                        )

                # per-block logits: sum partials over the 39 f-rows, scaled by
                # ones_val (aux cols 2/3) to undo the KV/denominator scaling
                fps = opsp.tile([2, NPAIR], fp32, tag="opsum")
                nc.tensor.matmul(fps[:], aux_sb[:, 2:4], logblk[:], start=True, stop=True)
                nc.scalar.add(
                    logits_sb[:, bass.ds(i * NPAIR, NPAIR)], fps[:], aux_sb[0:2, 4:5]
                )

            if max_unroll >= n_blocks:
                for i in range(n_blocks):
                    block_body(i)
            else:
                tc.For_i_unrolled(0, n_blocks, 1, block_body, max_unroll=max_unroll)

            nc.sync.dma_start(out=out[:], in_=logits_sb[:])

    nc.compile()
    return nc


def _get_program():
    if "nc" not in _CACHE:
        _CACHE["nc"] = _build_program()
    return _CACHE["nc"]


def _prep_weights(Wq, bq, Wk, bk, Wv, bv, Wp, bp, emb_scale):
    # Weights quantized to int8; dequant scale (incl. the int8 emb dequant
    # scale and the KQK/KV fp16-range boost) rides in aux and is applied on
    # device. The v ones-column (softmax denominator) is stored as 127 and
    # its scale is undone by ones_val in the final reduction matmul.
    f32 = np.float32
    Wq, Wk, Wv = Wq.astype(f32), Wk.astype(f32), Wv.astype(f32)
    bq, bk, bv = bq.astype(f32), bk.astype(f32), bv.astype(f32)
    s_qk = max(float(np.abs(Wq).max()), float(np.abs(Wk).max())) / 127.0 or 1.0
    s_v = float(np.abs(Wv).max()) / 127.0 or 1.0
    a_qk = emb_scale * s_qk * KQK
    a_v = emb_scale * s_v * KV
    ones_val = 127.0 * emb_scale * s_v  # = dequantized ones-col value / KV

    Wqk_i8 = np.concatenate(
        [
            np.concatenate([np.round(Wq / s_qk), np.round(bq / a_qk)[None, :]], 0),
            np.concatenate([np.round(Wk / s_qk), np.round(bk / a_qk)[None, :]], 0),
        ],
        axis=1,
    )
    Wqk_i8 = np.clip(Wqk_i8, -127, 127).astype(np.int8)

    Wva_i8 = np.zeros((KA, UV), f32)
    for h in range(H):
        Wva_i8[:E, h * VA:h * VA + DH] = np.round(Wv[:, h * DH:(h + 1) * DH] / s_v)
        Wva_i8[E, h * VA:h * VA + DH] = np.round(bv[h * DH:(h + 1) * DH] / a_v)
        Wva_i8[E, h * VA + DH] = 127.0
    Wva_i8 = np.clip(Wva_i8, -127, 127).astype(np.int8)
    W2 = np.concatenate([Wqk_i8, Wva_i8], axis=1)

    WpM = np.zeros((F, UV), f32)
    wp3 = Wp.astype(f32).reshape(F, H, DH)  # idx f*256 + h*32 + d
    for h in range(H):
        WpM[:, h * VA:h * VA + DH] = wp3[:, h, :]

    aux = np.zeros((128, 6), f32)
    aux[:, 0] = a_qk
    aux[:, 1] = a_v
    aux[0:F, 2] = ones_val
    aux[64:64 + F, 3] = ones_val
    aux[0:2, 4] = f32(bp[0])

    # byte-pack [aux | Wp | W2] (appended after the emb bytes by the caller)
    wbytes = np.empty(AUX_BYTES + WP_BYTES + W2_BYTES, np.int8)
    wbytes[0:AUX_BYTES] = np.ascontiguousarray(aux).view(np.int8).ravel()
    wbytes[AUX_BYTES:AUX_BYTES + WP_BYTES] = (
        np.ascontiguousarray(WpM.astype(np.float16)).view(np.int8).ravel()
    )
    wbytes[AUX_BYTES + WP_BYTES:] = np.ascontiguousarray(W2).ravel()
    return wbytes


def _pack6(q):
    # q [E, T] int6 values (-31..31), T % 8 == 0 -> [E, T//8*6] int8 bitstream
    u = (q.astype(np.int16) & 0x3F).astype(np.uint64)
    g = u.reshape(q.shape[0], -1, 8)
    bits = np.zeros(g.shape[:2], dtype=np.uint64)
    for j in range(8):
        bits |= g[:, :, j] << (6 * j)
    out = np.empty((*bits.shape, GB), dtype=np.uint8)
    for k in range(GB):
        out[:, :, k] = (bits >> (8 * k)) & 0xFF
    return out.reshape(q.shape[0], -1).view(np.int8)


def _make_in_maps(feature_ids, emb_table, Wq, bq, Wk, bk, Wv, bv, Wp, bp):
    feature_ids = np.asarray(feature_ids)
    emb_table = np.asarray(emb_table, dtype=np.float32)
    # clipped symmetric quantizer: ~3.3 sigma clip halves the step size vs
    # absmax at negligible clipping error
    clip_val = float(np.quantile(np.abs(emb_table), CLIP_Q))
    emb_scale = clip_val / QMAX
    if emb_scale == 0.0:
        emb_scale = 1.0
    table_q = np.clip(
        np.round(emb_table / emb_scale), -QMAX, QMAX
    ).astype(np.int8)
    wbytes = _prep_weights(
        np.asarray(Wq), np.asarray(bq), np.asarray(Wk), np.asarray(bk),
        np.asarray(Wv), np.asarray(bv), np.asarray(Wp), np.asarray(bp),
        emb_scale,
    )
    emb_bytes, _, _, _, nb_all = _blob_offsets(NBLK)
    in_maps = []
    for c in range(NCORES):
        ids_c = feature_ids[c * BC:(c + 1) * BC].astype(np.int64)   # [512, 39]
        emb_c = table_q[ids_c]                                      # [512, 39, 64]
        embT = np.ascontiguousarray(emb_c.reshape(T, E).T)          # [64, 19968]
        embP = _pack6(embT)                                         # [64, 14976]
        blob = np.empty((1, nb_all), np.int8)
        blob[0, 0:emb_bytes] = embP.ravel()
        blob[0, emb_bytes:] = wbytes
        in_maps.append({"IN": blob})
    return in_maps


def kernel(feature_ids, emb_table, Wq, bq, Wk, bk, Wv, bv, Wp, bp):
    from concourse.bass_utils import run_bass_kernel_spmd

    in_maps = _make_in_maps(
        feature_ids, emb_table, Wq, bq, Wk, bk, Wv, bv, Wp, bp
    )
    _CACHE["last_in_maps"] = in_maps

    nc = _get_program()
    res = run_bass_kernel_spmd(nc, in_maps, list(range(NCORES)))

    logits = np.empty((B, 1), np.float32)
    for c in range(NCORES):
        o = np.asarray(res.results[c]["out"])                       # [2, 256]
        logits[c * BC:(c + 1) * BC, 0] = (
            o.reshape(2, NBLK, NPAIR).transpose(1, 2, 0).reshape(BC)
        )
    return logits
